# revision 3
# baseline (speedup 1.0000x reference)
"""Trainium2 Bass kernel for nn_BitSpikeMambaModel (embed -> bitlinear x2 -> LN -> bitlinear head).

Self-contained: hardcodes shapes from the problem spec.
Sharding: pure data-parallel over the 4096 tokens (512 tokens per core, 8 cores).

Strategy vs the previous version:
  - BitNet ternary quantization done ON HOST (exactly mirroring the jax fp32
    reference: scale = max(mean|w|, eps) via jax-cpu when available), shipping
    ternary weights as fp16/fp8 (exact values in {-1,0,+1}).
  - scale folding: out2 = s0*s1*(q1@q0@x + bias_fold); LN is invariant to a
    scalar multiply, so the device never applies s0/s1 -- LN just uses
    eps' = eps/(s0*s1)^2 and bias_fold = (q1@b0)/s0 + b1/(s0*s1).
  - trunk matmuls in fp16 single precision (error ~1e-4 << 2e-2 tolerance).
  - head: fp16 single (mode 'f16'), or fp8e4 DoubleRow with hi/lo packed into
    the doubled contraction (modes 'dr'/'drb'/'drsw') -- out = q@(hi+lo).
Output per core: [V, T] (vocab-major); host reassembles/transposes to [2, S, V].
"""

import numpy as np

import concourse.bass as bass
import concourse.bacc as bacc
import concourse.mybir as mybir
import concourse.tile as tile
from concourse.bass_utils import run_bass_kernel_spmd

F32 = mybir.dt.float32
F16 = mybir.dt.float16
F8 = mybir.dt.float8e4
I16 = mybir.dt.int16
AF = mybir.ActivationFunctionType
OP = mybir.AluOpType
AX = mybir.AxisListType
PM = mybir.MatmulPerfMode
PE_ENG = mybir.EngineType.PE


def _dedup_ldweights(nc):
    """Drop consecutive identical LDWEIGHTS on the PE queue (the HW keeps the
    stationary operand loaded across non-self-loading matmuls); migrate their
    semaphore waits/updates to the next kept PE instruction."""
    import json
    n_drop = 0
    for blk in nc.main_func.blocks:
        insts = list(blk.instructions)
        drop_idx = []
        last_key = None
        pend_w, pend_u = [], []
        for i, inst in enumerate(insts):
            if getattr(inst, "engine", None) != PE_ENG:
                continue
            if type(inst).__name__ == "InstLdweights":
                j = json.loads(mybir.instruction_to_pretty_json_string(inst))
                for fld in ("name", "sync_info", "descendants", "debug",
                            "bass_addl_debug", "bass_priority",
                            "bass_scheduled_tick", "bass_scheduled_proc",
                            "bass_scheduled_scope", "bass_wait_until_ts"):
                    j.pop(fld, None)
                key = json.dumps(j, sort_keys=True)
                if key == last_key:
                    si = inst.sync_info
                    if si is not None:
                        pend_w.extend(list(si.on_wait))
                        pend_u.extend(list(si.on_update))
                    drop_idx.append(i)
                    n_drop += 1
                    continue
                last_key = key
            if pend_w or pend_u:
                si = inst.sync_info
                ow = list(si.on_wait) if si is not None else []
                ou = list(si.on_update) if si is not None else []
                inst.sync_info = mybir.SyncInfo(on_wait=ow + pend_w,
                                                on_update=ou + pend_u)
                pend_w, pend_u = [], []
        assert not pend_w and not pend_u, "trailing pended waits"
        for i in reversed(drop_idx):
            del blk.instructions[i]
    return n_drop

VOCAB = 32000
DIM = 2048
BATCH = 2
SEQ = 2048
NCORES = 8
EPS = 1e-5


class Cfg:
    def __init__(self, V=VOCAB, D=DIM, T=(BATCH * SEQ) // NCORES, ncores=NCORES,
                 head_mode="f16", g_tr=4, g_hd=2, hd_bufs=3, gather_split=4,
                 gb_trivial=True, sw_b=5, sw_dup=False, sw_dedup=True,
                 sw_wbufs=7, trunk_dr=False):
        assert D % 128 == 0 and V % 128 == 0 and T % 128 == 0 and T <= 512
        self.V, self.D, self.T, self.ncores = V, D, T, ncores
        self.head_mode = head_mode  # f16 | dr | drb | drsw | sw
        self.DT = D // 128          # d-tiles (contraction)
        self.NO_TR = D // 128       # trunk output tiles
        self.NO_HD = V // 128       # head output tiles
        self.G_TR = g_tr            # trunk o-tiles per weight DMA
        self.G_HD = g_hd            # head o-tiles per weight DMA
        self.HD_BUFS = hd_bufs
        self.GS = gather_split      # split embedding gather along d
        self.gb_trivial = gb_trivial  # ln_gamma==1 and ln_beta==0
        self.SW_B = sw_b            # sw head: vocab blocks per psum group
        self.sw_dup = sw_dup        # sw head: duplicate weights via 2x DMA
        self.sw_dedup = sw_dedup    # sw head: drop duplicate LDWEIGHTS
        self.SW_WBUFS = sw_wbufs    # sw head: weight stream tiles in flight
        self.trunk_dr = trunk_dr    # trunk via fp8 DoubleRow hi/lo
        assert self.NO_TR % self.G_TR == 0 and self.NO_HD % self.G_HD == 0
        assert self.DT % self.GS == 0


def build(cfg: Cfg):
    V, D, T, DT = cfg.V, cfg.D, cfg.T, cfg.DT
    hm = cfg.head_mode
    nc = bacc.Bacc("TRN2", target_bir_lowering=False, debug=False,
                   num_devices=cfg.ncores)

    # ---- DRAM I/O ----
    idx_d = nc.dram_tensor("idx", [128, T // 16], I16, kind="ExternalInput")
    if cfg.trunk_dr:
        # hi/lo interleaved fp8 table: embp[r, 2d]=hi, embp[r, 2d+1]=lo.
        # The transpose gather distributes 16-bit words, so partition p of
        # word-tile c receives exactly the (hi,lo) pair of d = c*128+p.
        embp_d = nc.dram_tensor("embp", [V, 2 * D], F8, kind="ExternalInput")
        q0p_d = nc.dram_tensor("q0p", [D, 2, D], F8, kind="ExternalInput")
        q1p_d = nc.dram_tensor("q1p", [D, 2, D], F8, kind="ExternalInput")
    else:
        embh_d = nc.dram_tensor("embh", [V, D], F16, kind="ExternalInput")
        q0t_d = nc.dram_tensor("q0t", [D, D], F16, kind="ExternalInput")
        q1t_d = nc.dram_tensor("q1t", [D, D], F16, kind="ExternalInput")
    if hm == "f16":
        hq_d = nc.dram_tensor("hqt", [D, V], F16, kind="ExternalInput")
    elif hm == "drsw":
        hq_d = nc.dram_tensor("hqsw", [D, 2 * V], F8, kind="ExternalInput")
    else:
        hq_d = nc.dram_tensor("hq8", [D, V], F8, kind="ExternalInput")
    bf_d = nc.dram_tensor("bfr", [128, DT], F32, kind="ExternalInput")
    gam_d = nc.dram_tensor("gamr", [128, DT], F32, kind="ExternalInput")
    bet_d = nc.dram_tensor("betr", [128, DT], F32, kind="ExternalInput")
    if hm != "sw":
        hb_d = nc.dram_tensor("hbr", [128, cfg.NO_HD], F32, kind="ExternalInput")
    sh_d = nc.dram_tensor("shr", [128, 1], F32, kind="ExternalInput")
    ep_d = nc.dram_tensor("epsr", [128, 1], F32, kind="ExternalInput")
    if hm == "sw":
        out_d = nc.dram_tensor("out", [T, V], F32, kind="ExternalOutput")
    else:
        out_d = nc.dram_tensor("out", [V, T], F32, kind="ExternalOutput")

    if cfg.trunk_dr:
        q0t_v = q0p_d.ap().rearrange("(dt p) two o -> p dt two o", p=128)
        q1t_v = q1p_d.ap().rearrange("(dt p) two o -> p dt two o", p=128)
    else:
        q0t_v = q0t_d.ap().rearrange("(dt p) o -> p dt o", p=128)
        q1t_v = q1t_d.ap().rearrange("(dt p) o -> p dt o", p=128)
    if hm == "drsw":
        hq_v = hq_d.ap().rearrange("(dt p) o -> p dt o", p=128)  # o in 2V units
    else:
        hq_v = hq_d.ap().rearrange("(dt p) o -> p dt o", p=128)

    with tile.TileContext(nc) as tc:
        import contextlib
        with contextlib.ExitStack() as ctx:
            cst = ctx.enter_context(tc.tile_pool(name="cst", bufs=1))
            act = ctx.enter_context(tc.tile_pool(name="act", bufs=1))
            wtr = ctx.enter_context(tc.tile_pool(name="wtr", bufs=2))
            whd = ctx.enter_context(tc.tile_pool(name="whd", bufs=cfg.HD_BUFS))
            evt = ctx.enter_context(tc.tile_pool(name="evt", bufs=2))
            osb = ctx.enter_context(tc.tile_pool(name="osb", bufs=4))
            sml = ctx.enter_context(tc.tile_pool(name="sml", bufs=1))
            mm_bufs = max(4, cfg.SW_B) if hm == "sw" else max(4, cfg.G_HD)
            ps_mm = ctx.enter_context(
                tc.tile_pool(name="ps_mm", bufs=mm_bufs, space="PSUM"))
            ps_st = ctx.enter_context(tc.tile_pool(name="ps_st", bufs=1, space="PSUM"))

            # ---- constants / small inputs ----
            ones_col = cst.tile([128, 1], F32)
            nc.any.memset(ones_col[:], 1.0)
            ones_row = cst.tile([1, 128], F32)
            nc.any.memset(ones_row[:], 1.0)
            idx_sb = cst.tile([128, T // 16], I16)
            nc.sync.dma_start(idx_sb[:], idx_d.ap())
            bfs = cst.tile([128, DT], F32)
            nc.sync.dma_start(bfs[:], bf_d.ap())
            gams = cst.tile([128, DT], F32)
            nc.sync.dma_start(gams[:], gam_d.ap())
            bets = cst.tile([128, DT], F32)
            nc.sync.dma_start(bets[:], bet_d.ap())
            if hm != "sw":
                hbs = cst.tile([128, cfg.NO_HD], F32)
                nc.sync.dma_start(hbs[:], hb_d.ap())
            shs = cst.tile([128, 1], F32)
            nc.sync.dma_start(shs[:], sh_d.ap())
            epsp = cst.tile([128, 1], F32)
            nc.sync.dma_start(epsp[:], ep_d.ap())

            # ---- embedding gather (transpose mode) ----
            # split along d so layer-0 matmuls (dt-outer) start after chunk 0
            dchunk = D // cfg.GS
            ndt = DT // cfg.GS
            if cfg.trunk_dr:
                # gather word layout per partition: byte f = c*2T + 2u + b holds
                # (b==0 ? hi : lo)[token_u, d=c*128+p]; keep the tile flat and
                # carve the DR pair AP [p, b, u] per word-tile c out of it.
                xt2 = act.tile([128, 2 * DT * T], F8, tag="xt")
                for k in range(cfg.GS):
                    nc.gpsimd.dma_gather(
                        out_ap=xt2[:, k * 2 * ndt * T:(k + 1) * 2 * ndt * T]
                        .rearrange("p (x t) -> p x t", x=2 * ndt),
                        in_ap=embp_d.ap()[:, k * 2 * dchunk:(k + 1) * 2 * dchunk],
                        idxs_ap=idx_sb[:], num_idxs=T, num_idxs_reg=T,
                        elem_size=2 * dchunk, elem_step=2 * D, transpose=True)

                def xt2_rhs(dt):
                    return (xt2[:, dt * 2 * T:(dt + 1) * 2 * T]
                            .rearrange("p (t b) -> p b t", b=2))
            else:
                xt = act.tile([128, DT, T], F16, tag="xt")
                for k in range(cfg.GS):
                    nc.gpsimd.dma_gather(
                        out_ap=xt[:, k * ndt:(k + 1) * ndt, :],
                        in_ap=embh_d.ap()[:, k * dchunk:(k + 1) * dchunk],
                        idxs_ap=idx_sb[:], num_idxs=T, num_idxs_reg=T,
                        elem_size=dchunk, elem_step=D, transpose=True)

            # ---- generic streamed ternary matmul layer (dt-outer order) ----
            def layer(wview, n_otiles, G, rhs, consume, wdt=F16, dma_split=1):
                for g in range(n_otiles // G):
                    wt = wtr.tile([128, DT, G * 128], wdt, tag="wtr")
                    ds = dma_split if g == 0 else 1
                    for k in range(ds):
                        nc.sync.dma_start(
                            wt[:, k * (DT // ds):(k + 1) * (DT // ds), :],
                            wview[:, k * (DT // ds):(k + 1) * (DT // ds),
                                  g * G * 128:(g + 1) * G * 128])
                    pts = [ps_mm.tile([128, T], F32, tag="ps_mm", name=f"pt{j}")
                           for j in range(G)]
                    for dt in range(DT):
                        for j in range(G):
                            nc.tensor.matmul(pts[j][:],
                                             wt[:, dt, j * 128:(j + 1) * 128],
                                             rhs[:, dt, :],
                                             start=(dt == 0), stop=(dt == DT - 1))
                    for j in range(G):
                        consume(g * G + j, pts[j])

            # DoubleRow trunk layer: weights pre-paired (q,q), rhs pairs (hi,lo)
            def layer_dr(wview, n_otiles, G, rhs_of, consume, dma_split=1):
                for g in range(n_otiles // G):
                    wt = wtr.tile([128, DT, 2, G * 128], F8, tag="wtr8")
                    ds = dma_split if g == 0 else 1
                    for k in range(ds):
                        for s in range(2):
                            nc.sync.dma_start(
                                wt[:, k * (DT // ds):(k + 1) * (DT // ds), s, :],
                                wview[:, k * (DT // ds):(k + 1) * (DT // ds), s,
                                      g * G * 128:(g + 1) * G * 128])
                    pts = [ps_mm.tile([128, T], F32, tag="ps_mm", name=f"pt{j}")
                           for j in range(G)]
                    for dt in range(DT):
                        rhs = rhs_of(dt)
                        for j in range(G):
                            nc.tensor.matmul(pts[j][:],
                                             wt[:, dt, :, j * 128:(j + 1) * 128],
                                             rhs,
                                             start=(dt == 0), stop=(dt == DT - 1),
                                             perf_mode=PM.DoubleRow)
                    for j in range(G):
                        consume(g * G + j, pts[j])

            # ---- layer 0: u = q0 @ x  (no scale/bias needed: folded) ----
            if cfg.trunk_dr:
                u2 = act.tile([128, 2, DT, T], F8, tag="u16")

                def consume_l0(ot, pt):
                    nc.scalar.activation(u2[:, 0, ot, :], pt[:], AF.Copy)
                    nc.vector.tensor_tensor(u2[:, 1, ot, :], pt[:],
                                            u2[:, 0, ot, :], OP.subtract)

                layer_dr(q0t_v, cfg.NO_TR, cfg.G_TR, xt2_rhs, consume_l0,
                         dma_split=cfg.GS)
            else:
                u16 = act.tile([128, DT, T], F16, tag="u16")

                def consume_l0(ot, pt):
                    nc.scalar.activation(u16[:, ot, :], pt[:], AF.Copy)

                layer(q0t_v, cfg.NO_TR, cfg.G_TR, xt, consume_l0,
                      dma_split=cfg.GS)

            # ---- layer 1: v = q1 @ u + bias_fold (fp32), LN stats on the fly ----
            y = act.tile([128, DT, T], F32, tag="y")
            ps_s = ps_st.tile([1, T], F32, tag="ps_s")
            ps_q = ps_st.tile([1, T], F32, tag="ps_q")

            def consume_l1(ot, pt):
                nc.scalar.activation(y[:, ot, :], pt[:], AF.Identity,
                                     bias=bfs[:, ot:ot + 1])
                sq = evt.tile([128, T], F32, tag="evt")
                nc.vector.tensor_tensor(sq[:], y[:, ot, :], y[:, ot, :], OP.mult)
                nc.tensor.matmul(ps_s[:], ones_col[:], y[:, ot, :],
                                 start=(ot == 0), stop=(ot == DT - 1))
                nc.tensor.matmul(ps_q[:], ones_col[:], sq[:],
                                 start=(ot == 0), stop=(ot == DT - 1))

            if cfg.trunk_dr:
                layer_dr(q1t_v, cfg.NO_TR, cfg.G_TR,
                         lambda dt: u2[:, :, dt, :], consume_l1)
            else:
                layer(q1t_v, cfg.NO_TR, cfg.G_TR, u16, consume_l1)

            # ---- layernorm: broadcast sums to [128,T] first (128-wide DVE),
            #      then rstd via sqrt+recip+1 Newton step ----
            s_row = sml.tile([1, T], F32, tag="s_row")
            nc.scalar.activation(s_row[:], ps_s[:], AF.Copy, scale=1.0 / D)
            q_row = sml.tile([1, T], F32, tag="q_row")
            nc.scalar.activation(q_row[:], ps_q[:], AF.Copy, scale=1.0 / D)
            pa = ps_st.tile([128, T], F32, tag="pa")
            nc.tensor.matmul(pa[:], ones_row[:], s_row[:], start=True, stop=True)
            mu_b = sml.tile([128, T], F32, tag="mu_b")
            nc.scalar.activation(mu_b[:], pa[:], AF.Copy)
            pb = ps_st.tile([128, T], F32, tag="pa", name="pb")
            nc.tensor.matmul(pb[:], ones_row[:], q_row[:], start=True, stop=True)
            ms_b = sml.tile([128, T], F32, tag="ms_b")
            nc.scalar.activation(ms_b[:], pb[:], AF.Copy)
            var = sml.tile([128, T], F32, tag="var")
            nc.vector.tensor_tensor(var[:], mu_b[:], mu_b[:], OP.mult)
            nc.vector.tensor_tensor(var[:], ms_b[:], var[:], OP.subtract)
            sd = sml.tile([128, T], F32, tag="sd")
            nc.scalar.activation(sd[:], var[:], AF.Sqrt, bias=epsp[:])
            a_b = sml.tile([128, T], F32, tag="a_b")
            nc.vector.reciprocal(a_b[:], sd[:])
            b_b = sml.tile([128, T], F32, tag="b_b")
            nc.vector.tensor_tensor(b_b[:], mu_b[:], a_b[:], OP.mult)
            nc.vector.tensor_scalar(b_b[:], b_b[:], -1.0, None, OP.mult)

            # ---- h3 = LN(v)*gamma + beta, packed for the head ----
            if hm == "f16":
                h3 = act.tile([128, DT, T], F16, tag="h3")
            else:
                h3 = act.tile([128, DT, 2, T], F8, tag="h3")
            bb16 = None
            if cfg.gb_trivial and hm == "f16":
                bb16 = sml.tile([128, T], F16, tag="bb16")
                nc.vector.tensor_copy(bb16[:], b_b[:])
            for dt in range(DT):
                if cfg.gb_trivial and hm == "f16":
                    m16 = evt.tile([128, T], F16, tag="evt16")
                    nc.vector.tensor_tensor(m16[:], y[:, dt, :], a_b[:], OP.mult)
                    nc.vector.tensor_tensor(h3[:, dt, :], m16[:], bb16[:], OP.add)
                    continue
                t1 = evt.tile([128, T], F32, tag="evt")
                nc.vector.tensor_tensor(t1[:], y[:, dt, :], a_b[:], OP.mult)
                nc.vector.tensor_tensor(t1[:], t1[:], b_b[:], OP.add)
                if not cfg.gb_trivial:
                    nc.vector.tensor_scalar(t1[:], t1[:], gams[:, dt:dt + 1],
                                            bets[:, dt:dt + 1], OP.mult, OP.add)
                if hm == "f16":
                    nc.vector.tensor_copy(h3[:, dt, :], t1[:])
                else:
                    # spread hi-cast (ScalarE) and lo-sub (GpSimd) off the DVE
                    nc.scalar.activation(h3[:, dt, 0, :], t1[:], AF.Copy)
                    nc.gpsimd.tensor_tensor(h3[:, dt, 1, :], t1[:], h3[:, dt, 0, :],
                                            OP.subtract)

            # ---- head (sw): stationary = h3 hi/lo pairs reused across vocab,
            #      moving = fp8 ternary weights (0-stride pair broadcast) ----
            if hm == "sw":
                vbs = []
                off = 0
                while off < V:
                    nv = min(512, V - off)
                    vbs.append((off, nv))
                    off += nv
                B = cfg.SW_B
                for gs in range(0, len(vbs), B):
                    grp = vbs[gs:gs + B]
                    wts = []
                    for i, (off, nv) in enumerate(grp):
                        if cfg.sw_dup:
                            wv = whd.tile([128, DT, 2, 512], F8, tag="whd",
                                          name=f"wv{i}", bufs=cfg.SW_WBUFS)
                            nc.sync.dma_start(wv[:, :, 0, :nv],
                                              hq_v[:, :, off:off + nv])
                            nc.sync.dma_start(wv[:, :, 1, :nv],
                                              hq_v[:, :, off:off + nv])
                        else:
                            wv = whd.tile([128, DT, 512], F8, tag="whd",
                                          name=f"wv{i}", bufs=cfg.SW_WBUFS)
                            nc.sync.dma_start(wv[:, :, :nv],
                                              hq_v[:, :, off:off + nv])
                        wts.append(wv)
                    for tb in range(T // 128):
                        hpts = [ps_mm.tile([128, 512], F32, tag="ps_mm",
                                           name=f"hpt{i}") for i in range(len(grp))]
                        for dt in range(DT):
                            lhsT = h3[:, dt, :, tb * 128:(tb + 1) * 128]
                            for i, (off, nv) in enumerate(grp):
                                if cfg.sw_dup:
                                    rhs = wts[i][:, dt, :, :nv]
                                else:
                                    rhs = (wts[i][:, dt, :nv].unsqueeze(1)
                                           .broadcast_to([128, 2, nv]))
                                nc.tensor.matmul(hpts[i][:, :nv], lhsT, rhs,
                                                 start=(dt == 0),
                                                 stop=(dt == DT - 1),
                                                 perf_mode=PM.DoubleRow)
                        for i, (off, nv) in enumerate(grp):
                            o = osb.tile([128, 512], F32, tag="osb")
                            nc.scalar.activation(o[:, :nv], hpts[i][:, :nv],
                                                 AF.Copy, scale=shs[:])
                            nc.sync.dma_start(
                                out_d.ap()[tb * 128:(tb + 1) * 128, off:off + nv],
                                o[:, :nv])

            def consume_head(ot, pt):
                o = osb.tile([128, T], F32, tag="osb")
                nc.scalar.activation(o[:], pt[:], AF.Identity,
                                     bias=hbs[:, ot:ot + 1], scale=shs[:])
                nc.sync.dma_start(out_d.ap()[ot * 128:(ot + 1) * 128, :], o[:])

            G = cfg.G_HD
            if hm == "sw":
                pass  # handled above
            elif hm == "f16":
                layer(hq_v, cfg.NO_HD, G, h3, consume_head)
            else:
                for g in range(cfg.NO_HD // G):
                    if hm == "dr":
                        wt = whd.tile([128, DT, 2, G * 128], F8, tag="whd")
                        sl = hq_v[:, :, g * G * 128:(g + 1) * G * 128]
                        nc.sync.dma_start(wt[:, :, 0, :], sl)
                        nc.sync.dma_start(wt[:, :, 1, :], sl)
                    elif hm == "drb":
                        wt = whd.tile([128, DT, G * 128], F8, tag="whd")
                        nc.sync.dma_start(
                            wt[:], hq_v[:, :, g * G * 128:(g + 1) * G * 128])
                    elif hm == "drsw":
                        wt = whd.tile([128, DT, G * 256], F8, tag="whd")
                        nc.sync.dma_start(
                            wt[:], hq_v[:, :, g * G * 256:(g + 1) * G * 256])
                    for j in range(G):
                        ot = g * G + j
                        pt = ps_mm.tile([128, T], F32, tag="ps_mm")
                        for dt in range(DT):
                            if hm == "dr":
                                lhsT = wt[:, dt, :, j * 128:(j + 1) * 128]
                                pm = PM.DoubleRow
                            elif hm == "drb":
                                lhsT = (wt[:, dt, j * 128:(j + 1) * 128]
                                        .unsqueeze(1).broadcast_to([128, 2, 128]))
                                pm = PM.DoubleRow
                            else:  # drsw
                                lhsT = wt[:, dt, j * 256:(j + 1) * 256]
                                pm = PM.DoubleRowSwInterleave
                            nc.tensor.matmul(pt[:], lhsT, h3[:, dt, :, :],
                                             start=(dt == 0), stop=(dt == DT - 1),
                                             perf_mode=pm)
                        consume_head(ot, pt)

    nc.compile()
    if cfg.sw_dedup:
        _dedup_ldweights(nc)
    return nc


_BUILD_CACHE = {}


def _get_nc(cfg: Cfg):
    key = (cfg.V, cfg.D, cfg.T, cfg.ncores, cfg.head_mode, cfg.G_TR, cfg.G_HD,
           cfg.HD_BUFS, cfg.GS, cfg.gb_trivial, cfg.SW_B, cfg.sw_dup,
           cfg.sw_dedup, cfg.SW_WBUFS, cfg.trunk_dr)
    if key not in _BUILD_CACHE:
        _BUILD_CACHE[key] = build(cfg)
    return _BUILD_CACHE[key]


def _scale_of(w):
    """max(mean(|w|), EPS) in fp32, mirroring the jax reference as closely as
    possible (jax-cpu when importable, else numpy pairwise fp32)."""
    try:
        import jax
        import jax.numpy as jnp
        cpu = jax.devices("cpu")[0]
        with jax.default_device(cpu):
            s = jnp.maximum(jnp.mean(jnp.abs(jnp.asarray(w, jnp.float32))),
                            jnp.float32(EPS))
            return np.float32(s)
    except Exception:
        return np.float32(max(np.mean(np.abs(w), dtype=np.float32), EPS))


def _ternary(w, s):
    """clip(round_half_even(w/s), -1, 1) elementwise in fp32 (exact mirror)."""
    ws = (np.asarray(w, np.float32) / np.float32(s)).astype(np.float32)
    return np.clip(np.rint(ws), -1.0, 1.0).astype(np.float32)


def make_in_maps(cfg: Cfg, x, emb, w0, b0, w1, b1, ln_gamma, ln_beta, head_w, head_b):
    """Host-side quantization + sharding/layout prep -> per-core input dicts."""
    import ml_dtypes
    V, D, T = cfg.V, cfg.D, cfg.T
    f8 = ml_dtypes.float8_e4m3

    s0 = _scale_of(w0)
    s1 = _scale_of(w1)
    sh = _scale_of(head_w)
    q0 = _ternary(w0, s0)
    q1 = _ternary(w1, s1)
    qh = _ternary(head_w, sh)

    emb32 = np.asarray(emb, np.float32)
    if cfg.trunk_dr:
        emb8h = emb32.astype(f8)
        emb8l = (emb32 - emb8h.astype(np.float32)).astype(f8)
        embp = np.empty((V, 2 * D), f8)
        embp[:, 0::2] = emb8h
        embp[:, 1::2] = emb8l
        q0p = np.repeat(np.ascontiguousarray(q0.T)[:, None, :], 2,
                        axis=1).astype(f8)
        q1p = np.repeat(np.ascontiguousarray(q1.T)[:, None, :], 2,
                        axis=1).astype(f8)
    else:
        embh = emb32.astype(np.float16)
        q0t = np.ascontiguousarray(q0.T).astype(np.float16)
        q1t = np.ascontiguousarray(q1.T).astype(np.float16)
    qht = np.ascontiguousarray(qh.T)  # [D, V]
    if cfg.head_mode == "f16":
        hq = qht.astype(np.float16)
        hq_name = "hqt"
    elif cfg.head_mode == "drsw":
        # per o-tile SwInterleave layout: each 128-col block becomes 256 cols
        # [A127 B127 A126 B126 ... A0 B0] with A=B=q (duplicated, col-reversed)
        blk = qht.reshape(D, cfg.NO_HD, 128)[:, :, ::-1]          # reversed cols
        hq = np.repeat(blk, 2, axis=2).reshape(D, 2 * V).astype(f8)
        hq_name = "hqsw"
    else:
        hq = qht.astype(f8)
        hq_name = "hq8"

    bias_fold = ((q1.astype(np.float64) @ np.asarray(b0, np.float64)) / float(s0)
                 + np.asarray(b1, np.float64) / (float(s0) * float(s1)))
    eps_p = np.float32(EPS / (float(s0) * float(s1)) ** 2)

    def rearr(v, n):
        return np.ascontiguousarray(np.asarray(v, np.float32).reshape(n, 128).T)

    bfr = rearr(bias_fold, D // 128)
    gamr = rearr(ln_gamma, D // 128)
    betr = rearr(ln_beta, D // 128)
    hbr = rearr(head_b, V // 128)
    shr = np.full((128, 1), sh, np.float32)
    epsr = np.full((128, 1), eps_p, np.float32)

    ids = np.asarray(x).reshape(-1).astype(np.int16)
    assert ids.size == cfg.ncores * T
    in_maps = []
    for c in range(cfg.ncores):
        # indices wrapped into 16 partitions, replicated across the 8 Q7 stripes
        idx_arr = np.tile(ids[c * T:(c + 1) * T].reshape(T // 16, 16).T, (8, 1))
        m = {
            "idx": idx_arr, hq_name: hq,
            "bfr": bfr, "gamr": gamr, "betr": betr, "shr": shr, "epsr": epsr}
        if cfg.trunk_dr:
            m.update(embp=embp, q0p=q0p, q1p=q1p)
        else:
            m.update(embh=embh, q0t=q0t, q1t=q1t)
        if cfg.head_mode != "sw":
            m["hbr"] = hbr
        in_maps.append(m)
    return in_maps


def _run(cfg: Cfg, inputs, trace=False):
    nc = _get_nc(cfg)
    in_maps = make_in_maps(cfg, **inputs)
    res = run_bass_kernel_spmd(nc, in_maps, core_ids=list(range(cfg.ncores)),
                               trace=trace)
    if cfg.head_mode == "sw":
        outs = [res.results[c]["out"].reshape(cfg.T, cfg.V)
                for c in range(cfg.ncores)]
        full = np.concatenate(outs, axis=0)  # [ncores*T, V]
        full += np.asarray(inputs["head_b"], np.float32)[None, :]
    else:
        outs = [res.results[c]["out"].reshape(cfg.V, cfg.T)
                for c in range(cfg.ncores)]
        full = np.concatenate([o.T for o in outs], axis=0)  # [ncores*T, V]
    return full, res


def kernel(**inputs) -> np.ndarray:
    gb_trivial = bool(np.all(np.asarray(inputs["ln_gamma"]) == 1.0)
                      and np.all(np.asarray(inputs["ln_beta"]) == 0.0))
    cfg = Cfg(gb_trivial=gb_trivial)
    full, _ = _run(cfg, inputs)
    return full.reshape(BATCH, SEQ, VOCAB)
